# revision 39
# baseline (speedup 1.0000x reference)
import sys
import numpy as np
import ml_dtypes

sys.path.insert(0, "/opt/trn_rl_repo")

from concourse import bass, bacc, tile, mybir
from concourse import bass_utils

BF16 = ml_dtypes.bfloat16
F8 = ml_dtypes.float8_e4m3
dt = mybir.dt



class Cfg:
    def __init__(self, NT=49, n_real=50000):
        self.NC = 8
        self.TP = 128
        self.NT = NT
        self.IN = 256
        self.HID = 256
        self.OUT = 128
        self.FUSED = 512
        self.SHARD = NT * self.TP
        self.NPAD = self.NC * self.SHARD
        self.n_real = n_real
        self.LN_EPS = 1e-5
        self.P1_CALLS = [384, 1024, 1024]
        self.P1_BASES = [None, 0, 17408]
        self.P2_CALLS = [1024, 640, 512]
        self.P2_BASES = [0, 8704, 17408]
        self.SLOTS = 2432
        self.CHUNKS = self.SLOTS // self.TP
        self.SLOTS2 = sum(self.P2_CALLS)
        self.CHUNKS2 = self.SLOTS2 // self.TP


FULL = Cfg()



def _wrap16(vals, nrows=128):
    n = vals.shape[0]
    assert n % 16 == 0
    w = vals.reshape(n // 16, 16).T
    return np.tile(w, (nrows // 16, 1))


def _pmaj(vals, TP=128):
    return np.ascontiguousarray(vals.reshape(-1, TP).T)


def _pack_calls(s_sorted, d_sorted, calls, bases):
    total = sum(calls)
    idx = np.zeros(total, np.int64)
    seg = -np.ones(total, np.float32)
    pos = 0
    off = 0
    n = s_sorted.shape[0]
    for cap, base in zip(calls, bases):
        take = 0
        limit = base + 32768
        while take < cap and pos + take < n and s_sorted[pos + take] < limit:
            take += 1
        s_blk = s_sorted[pos:pos + take]
        if take and s_blk[0] < base:
            raise OverflowError(f"src {s_blk[0]} below window base {base}")
        idx[off:off + take] = s_blk - base
        seg[off:off + take] = d_sorted[pos:pos + take]
        pos += take
        off += cap
    if pos != n:
        raise OverflowError(f"failed to pack: {n - pos} messages left over")
    return idx, seg


def preprocess(edge_index, cfg):
    src = np.asarray(edge_index[0], dtype=np.int64)
    dst = np.asarray(edge_index[1], dtype=np.int64)
    deg = np.bincount(src, minlength=cfg.NPAD).astype(np.float32) + 1.0

    per_core = []
    shard_of = dst // cfg.SHARD
    for c in range(cfg.NC):
        m = shard_of == c
        s_c = src[m]
        d_c = dst[m] - c * cfg.SHARD
        tile_of = d_c // cfg.TP
        order = np.argsort(tile_of, kind="stable")
        s_c, d_c, tile_of = s_c[order], d_c[order], tile_of[order]
        bounds = np.searchsorted(tile_of, np.arange(cfg.NT + 1))

        lo, hi = c * cfg.SHARD, (c + 1) * cfg.SHARD
        cap0 = cfg.P1_CALLS[0]
        idx1 = np.zeros((cfg.NT, cfg.SLOTS), np.int64)
        seg1 = -np.ones((cfg.NT, cfg.SLOTS), np.float32)
        idx2 = np.zeros((cfg.NT, cfg.SLOTS2), np.int64)
        seg2 = -np.ones((cfg.NT, cfg.SLOTS2), np.float32)
        for t in range(cfg.NT):
            sl = slice(bounds[t], bounds[t + 1])
            s_t = s_c[sl]
            d_t = d_c[sl] - t * cfg.TP
            o = np.argsort(s_t, kind="stable")
            s_t, d_t = s_t[o], d_t[o]

            own = (s_t >= lo) & (s_t < hi)
            s_o, d_o = s_t[own], d_t[own]
            n_o = s_o.shape[0]
            if n_o > cap0:
                raise OverflowError(f"own overflow {n_o}")
            idx1[t, :n_o] = s_o - lo
            seg1[t, :n_o] = d_o
            s_x, d_x = s_t[~own], d_t[~own]
            ix, sg = _pack_calls(s_x, d_x, cfg.P1_CALLS[1:], cfg.P1_BASES[1:])
            idx1[t, cap0:] = ix
            seg1[t, cap0:] = sg

            ix, sg = _pack_calls(s_t, d_t, cfg.P2_CALLS, cfg.P2_BASES)
            idx2[t] = ix
            seg2[t] = sg

        def segmaj(a):
            return np.ascontiguousarray(a.reshape(-1, cfg.TP).T)

        idx1r = np.concatenate([idx1[:, :cap0].reshape(-1),
                                idx1[:, cap0:].reshape(-1)])
        seg1r = np.concatenate([segmaj(seg1[:, :cap0]),
                                segmaj(seg1[:, cap0:])], axis=1)
        per_core.append({
            "idx1": _wrap16(idx1r).astype(np.int16),
            "idx2": _wrap16(idx2.reshape(-1)).astype(np.int16),
            "seg1": np.ascontiguousarray(seg1r).astype(BF16),
            "seg2": segmaj(seg2).astype(BF16),
        })
    return deg, per_core


def make_in_maps(inputs, cfg):
    x = np.asarray(inputs["x"], np.float32)
    deg, per_core = preprocess(np.asarray(inputs["edge_index"]), cfg)

    xp = np.zeros((cfg.NPAD, cfg.IN), np.float32)
    xp[: x.shape[0]] = x
    dinv = deg ** -0.5
    xf8 = (dinv[:, None] * xp).astype(F8)

    def chunks(a, k):
        return np.ascontiguousarray(a.reshape(k, 128, a.shape[1]))

    wf = np.concatenate([np.asarray(inputs["sig_conv_w"], np.float32),
                         np.asarray(inputs["conv1_w"], np.float32)], axis=1)

    def aug(w, b):
        wt = np.asarray(w, np.float32).T
        a = np.zeros((3 * 128, wt.shape[1]), np.float32)
        a[: wt.shape[0]] = wt
        a[wt.shape[0]] = np.asarray(b, np.float32)
        return chunks(a, 3)

    shared = {
        "wf": chunks(wf, 2).astype(BF16),
        "w2": chunks(np.asarray(inputs["conv2_w"], np.float32), 2).astype(BF16),
        "wg1": aug(inputs["fc1_w"], inputs["fc1_b"]),
        "wb1": aug(inputs["fc2_w"], inputs["fc2_b"]),
        "wg2": aug(inputs["fc3_w"], inputs["fc3_b"]),
        "wb2": aug(inputs["fc4_w"], inputs["fc4_b"]),
        "bsig": np.broadcast_to(np.asarray(inputs["sig_conv_b"], np.float32),
                                (128, cfg.HID)).copy(),
        "b1c": np.broadcast_to(np.asarray(inputs["conv1_b"], np.float32),
                               (128, cfg.HID)).copy(),
        "b2c": np.broadcast_to(np.asarray(inputs["conv2_b"], np.float32),
                               (128, cfg.OUT)).copy(),
        "iota": np.broadcast_to(np.arange(128, dtype=np.float32),
                                (128, 128)).astype(BF16).copy(),
        "ident": np.eye(128, dtype=np.float32).astype(BF16),
        "identf8": np.eye(128, dtype=np.float32).astype(F8),
    }

    in_maps = []
    node_ids = np.arange(cfg.SHARD)
    for c in range(cfg.NC):
        sl = slice(c * cfg.SHARD, (c + 1) * cfg.SHARD)
        gids = node_ids + c * cfg.SHARD
        m = dict(shared)
        m["xsh"] = np.ascontiguousarray(xf8[sl])
        m["xnp"] = np.ascontiguousarray(
            xf8[sl].reshape(cfg.NT, cfg.TP, cfg.IN).transpose(1, 0, 2))
        m["deg"] = _pmaj(deg[sl]).copy()
        m["sigmask"] = _pmaj((gids < cfg.n_real).astype(np.float32)).astype(BF16)
        m.update(per_core[c])
        in_maps.append(m)
    return in_maps



def build_program(cfg):
    nc = bacc.Bacc("TRN2", target_bir_lowering=False, debug=False,
                   num_devices=cfg.NC, num_swdge_queues=4)
    f32, bf16, i16, f8 = dt.float32, dt.bfloat16, dt.int16, dt.float8e4
    TP, NT = cfg.TP, cfg.NT
    IN, HID, OUT, FUSED = cfg.IN, cfg.HID, cfg.OUT, cfg.FUSED
    CH = cfg.CHUNKS
    ICOLS = cfg.SLOTS * NT // 16
    SCOLS = CH * NT

    def inp(name, shape, dtype):
        return nc.dram_tensor(name, shape, dtype, kind="ExternalInput")

    xsh_d = inp("xsh", [cfg.SHARD, IN], f8)
    xnp_d = inp("xnp", [TP, NT, IN], f8)
    wf_d = inp("wf", [2, TP, FUSED], bf16)
    w2_d = inp("w2", [2, TP, OUT], bf16)
    wg1_d = inp("wg1", [3, TP, HID], f32)
    wb1_d = inp("wb1", [3, TP, HID], f32)
    wg2_d = inp("wg2", [3, TP, OUT], f32)
    wb2_d = inp("wb2", [3, TP, OUT], f32)
    bsig_d = inp("bsig", [TP, HID], f32)
    b1c_d = inp("b1c", [TP, HID], f32)
    b2c_d = inp("b2c", [TP, OUT], f32)
    iota_d = inp("iota", [TP, TP], bf16)
    ident_d = inp("ident", [TP, TP], bf16)
    identf8_d = inp("identf8", [TP, TP], f8)
    deg_d = inp("deg", [TP, NT], f32)
    mask_d = inp("sigmask", [TP, NT], bf16)
    idx1_d = inp("idx1", [TP, ICOLS], i16)
    idx2_d = inp("idx2", [TP, cfg.SLOTS2 * NT // 16], i16)
    seg1_d = inp("seg1", [TP, SCOLS], bf16)
    seg2_d = inp("seg2", [TP, cfg.CHUNKS2 * NT], bf16)

    out_d = nc.dram_tensor("out", [cfg.SHARD, OUT], f32, kind="ExternalOutput")

    xin_d = nc.dram_tensor("xin", [cfg.SHARD, IN], f8)
    xfull_d = nc.dram_tensor("xfull", [cfg.NPAD, IN], f8,
                             addr_space="Shared")
    tsh_d = nc.dram_tensor("tsh", [cfg.SHARD, OUT], bf16)
    tfull_d = nc.dram_tensor("tfull", [cfg.NPAD, OUT], bf16,
                             addr_space="Shared")
    sin_d = nc.dram_tensor("sin", [1, HID], f32)
    sout_d = nc.dram_tensor("sout", [1, HID], f32, addr_space="Shared")

    rg = [list(range(cfg.NC))]
    qctr = [0]

    with tile.TileContext(nc) as tc:
        with (
            tc.tile_pool(name="const", bufs=1) as const,
            tc.tile_pool(name="persist", bufs=1) as persist,
            tc.tile_pool(name="gat", bufs=6) as gat,
            tc.tile_pool(name="gat2", bufs=4) as gat2,
            tc.tile_pool(name="gato", bufs=6) as gato,
            tc.tile_pool(name="sbuild", bufs=6) as sbuild,
            tc.tile_pool(name="sbuild2", bufs=4) as sbuild2,
            tc.tile_pool(name="epi", bufs=3) as epi,
            tc.tile_pool(name="small", bufs=4) as small,
            tc.tile_pool(name="enc_p", bufs=8) as enc_p,
            tc.tile_pool(name="one", bufs=1) as one,
            tc.tile_pool(name="ps_agg", bufs=3, space="PSUM") as ps_agg,
            tc.tile_pool(name="ps_big", bufs=2, space="PSUM") as ps_big,
            tc.tile_pool(name="ps_sig", bufs=1, space="PSUM") as ps_sig,
            tc.tile_pool(name="ps_sm", bufs=2, space="PSUM") as ps_sm,
        ):
            idxA_sb = const.tile([TP, ICOLS], i16)
            seg1_sb = const.tile([TP, SCOLS], bf16)
            seg2_sb = const.tile([TP, cfg.CHUNKS2 * NT], bf16)
            iota_sb = const.tile([TP, TP], bf16)
            ident_sb = const.tile([TP, TP], bf16)
            identf8_sb = const.tile([TP, TP], f8)
            deg_sb = const.tile([TP, NT], f32)
            mask_sb = const.tile([TP, NT], bf16)
            OCOLS = NT * cfg.P1_CALLS[0] // 16
            nc.sync.dma_start(out=xin_d.ap(), in_=xsh_d.ap())
            nc.sync.dma_start(out=idxA_sb[:, :OCOLS],
                              in_=idx1_d.ap()[:, :OCOLS])
            for t_, d in ((seg1_sb, seg1_d), (iota_sb, iota_d),
                          (identf8_sb, identf8_d), (deg_sb, deg_d)):
                nc.sync.dma_start(out=t_[:], in_=d.ap())
            xn_sb = persist.tile([TP, NT, IN], f8)
            ownp_sb = persist.tile([TP, NT, IN], bf16)
            c1agg_sb = persist.tile([TP, NT, HID], bf16)
            tp_sb = persist.tile([TP, NT, OUT], bf16)
            nc.sync.dma_start(out=xn_sb[:], in_=xnp_d.ap())
            nc.sync.dma_start(out=idxA_sb[:, OCOLS:],
                              in_=idx1_d.ap()[:, OCOLS:])

            wf_sb = const.tile([TP, 2, FUSED], bf16)
            w2_sb = const.tile([TP, 2, OUT], bf16)
            nc.sync.dma_start(out=wf_sb[:], in_=wf_d.ap().transpose([1, 0, 2]))
            nc.sync.dma_start(out=w2_sb[:], in_=w2_d.ap().transpose([1, 0, 2]))
            fc_sb = {}
            for nm, d, width in (("wg1", wg1_d, HID), ("wb1", wb1_d, HID),
                                 ("wg2", wg2_d, OUT), ("wb2", wb2_d, OUT)):
                t_ = const.tile([TP, 3, width], f32, name=nm)
                nc.sync.dma_start(out=t_[:], in_=d.ap().transpose([1, 0, 2]))
                fc_sb[nm] = t_
            bsig_sb = const.tile([TP, HID], f32)
            b1c_sb = const.tile([TP, HID], f32)
            b2c_sb = const.tile([TP, OUT], f32)
            for t_, d in ((bsig_sb, bsig_d), (b1c_sb, b1c_d),
                          (b2c_sb, b2c_d), (seg2_sb, seg2_d),
                          (mask_sb, mask_d), (ident_sb, ident_d)):
                nc.sync.dma_start(out=t_[:], in_=d.ap())

            eps_sb = const.tile([TP, 1], f32)
            nc.vector.memset(eps_sb[:], cfg.LN_EPS)
            ones_sb = const.tile([TP, 1], f32)
            nc.vector.memset(ones_sb[:], 1.0)
            dinv_sb = const.tile([TP, NT], f32)
            nc.scalar.sqrt(dinv_sb[:], deg_sb[:])
            nc.vector.reciprocal(dinv_sb[:], dinv_sb[:])

            def build_S(pool, seg_src, sdt, scol, nch, name):
                S = pool.tile([TP, nch, TP], sdt,
                              tag=f"S{nch}_{sdt}", name=name)
                cols = slice(scol, scol + nch)
                nc.vector.tensor_tensor(
                    S[:],
                    seg_src[:, cols].unsqueeze(2).to_broadcast((TP, nch, TP)),
                    iota_sb[:].unsqueeze(1).to_broadcast((TP, nch, TP)),
                    mybir.AluOpType.is_equal)
                return S

            def gather(pool, table_ap, icol0, n_idx, width, gdt, name):
                g = pool.tile([TP, n_idx // TP, width], gdt,
                              tag=f"g{n_idx}_{width}_{gdt}", name=name)
                nc.gpsimd.dma_gather(
                    out_ap=g[:],
                    in_ap=table_ap,
                    idxs_ap=idxA_sb[:, icol0:icol0 + n_idx // 16],
                    num_idxs=n_idx,
                    num_idxs_reg=n_idx,
                    elem_size=width,
                    queue_num=qctr[0] % 4,
                )
                qctr[0] += 1
                return g

            def own_tile(t):
                n_o = cfg.P1_CALLS[0]
                g = gather(gato, xsh_d.ap(), t * (n_o // 16), n_o, IN,
                           f8, f"go_{t}")
                S = build_S(sbuild, seg1_sb, f8, t * (n_o // TP),
                            n_o // TP, f"So_{t}")
                ps = ps_agg.tile([TP, IN], f32, tag="agg", name=f"po_{t}")
                nc.tensor.matmul(ps[:], identf8_sb[:], xn_sb[:, t, :],
                                 start=True, stop=False)
                for k in range(n_o // TP):
                    nc.tensor.matmul(ps[:], S[:, k, :], g[:, k, :],
                                     start=False,
                                     stop=(k == n_o // TP - 1))
                nc.scalar.copy(ownp_sb[:, t, :], ps[:])

            with nc.named_scope("own1"):
                for t in range(4):
                    own_tile(t)
            with nc.named_scope("ag1"):
                nc.gpsimd.collective_compute(
                    "AllGather", mybir.AluOpType.bypass, replica_groups=rg,
                    ins=[xin_d.ap().opt()], outs=[xfull_d.ap().opt()])
            with nc.named_scope("own1b"):
                for t in range(4, NT):
                    own_tile(t)

            s_ps = ps_sig.tile([1, HID], f32)
            with nc.named_scope("cross1"):
                for t in range(NT):
                    ps = ps_agg.tile([TP, IN], f32, tag="agg", name=f"pa_{t}")
                    n_own = cfg.P1_CALLS[0]
                    nco = sum(cfg.P1_CALLS[1:])
                    ib = NT * n_own // 16 + t * nco // 16
                    sb_ = NT * (n_own // TP) + t * (nco // TP)
                    slot0 = 0
                    c0 = 0
                    first = True
                    for ci, n_i in enumerate(cfg.P1_CALLS[1:]):
                        base = cfg.P1_BASES[1 + ci]
                        table = xfull_d.ap()[base:base + 32768, :]
                        g = gather(gat, table, ib + slot0 // 16, n_i, IN, f8,
                                   f"gx_{t}_{ci}")
                        S = build_S(sbuild, seg1_sb, f8, sb_ + c0,
                                    n_i // TP, f"Sx_{t}_{ci}")
                        for k in range(n_i // TP):
                            nc.tensor.matmul(
                                ps[:], S[:, k, :], g[:, k, :],
                                start=first,
                                stop=(ci == len(cfg.P1_CALLS) - 2
                                      and k == n_i // TP - 1))
                            first = False
                        slot0 += n_i
                        c0 += n_i // TP
                    agg_b = epi.tile([TP, IN], bf16, tag="aggb",
                                     name=f"ab_{t}")
                    nc.vector.tensor_tensor(agg_b[:], ps[:],
                                            ownp_sb[:, t, :],
                                            mybir.AluOpType.add)
                    aggT = epi.tile([TP, 2, TP], bf16, tag="aggT",
                                    name=f"at_{t}")
                    ps_t = ps_sm.tile([TP, 2, TP], bf16, tag="smt",
                                      name=f"tr_{t}")
                    for k in range(2):
                        nc.tensor.transpose(ps_t[:, k, :],
                                            agg_b[:, k * TP:(k + 1) * TP],
                                            ident_sb[:])
                    nc.scalar.copy(aggT[:], ps_t[:])
                    po = ps_big.tile([TP, FUSED], f32, tag="big",
                                     name=f"pw_{t}")
                    for k in range(2):
                        nc.tensor.matmul(po[:], aggT[:, k, :], wf_sb[:, k, :],
                                         start=(k == 0), stop=(k == 1))
                    dv = dinv_sb[:, t:t + 1]
                    sig_f = epi.tile([TP, HID], f32, tag="sigf",
                                     name=f"sf_{t}")
                    nc.vector.scalar_tensor_tensor(
                        sig_f[:], po[:, :HID], dv, bsig_sb[:],
                        mybir.AluOpType.mult, mybir.AluOpType.add)
                    sig_b = epi.tile([TP, HID], bf16, tag="sigb",
                                     name=f"sb_{t}")
                    nc.scalar.activation(sig_b[:], sig_f[:],
                                         mybir.ActivationFunctionType.Relu)
                    nc.tensor.matmul(s_ps[:], mask_sb[:, t:t + 1], sig_b[:],
                                     start=(t == 0), stop=(t == NT - 1))
                    nc.scalar.activation(c1agg_sb[:, t, :], po[:, HID:],
                                         mybir.ActivationFunctionType.Copy,
                                         scale=dv)

            with nc.named_scope("sig"):
                s_sb = one.tile([1, HID], f32)
                nc.scalar.copy(s_sb[:], s_ps[:])
                nc.sync.dma_start(out=sin_d.ap(), in_=s_sb[:])
                nc.gpsimd.collective_compute(
                    "AllReduce", mybir.AluOpType.add, replica_groups=rg,
                    ins=[sin_d.ap().opt()], outs=[sout_d.ap().opt()])
                nc.sync.dma_start(
                    out=idxA_sb[:, :cfg.SLOTS2 * NT // 16],
                    in_=idx2_d.ap())

                s_col = one.tile([TP, 3], f32)
                nc.vector.memset(s_col[:], 0.0)
                nc.vector.memset(s_col[0:1, 2:3], 1.0)
                nc.sync.dma_start(
                    out=s_col[:, 0:2],
                    in_=sout_d.ap().rearrange("o (c p) -> (o c) p", p=TP)
                        .transpose([1, 0]))
                s_rep = one.tile([TP, 3, TP], f32)
                for c in range(3):
                    nc.vector.tensor_copy(
                        s_rep[:, c, :],
                        s_col[:, c:c + 1].to_broadcast((TP, TP)))
                gb_sb = {}
                for nm, width in (("wg1", HID), ("wb1", HID),
                                  ("wg2", OUT), ("wb2", OUT)):
                    ps_fcw = ps_big.tile([TP, FUSED], f32, tag="big",
                                         name=nm)
                    ps_fc = ps_fcw[:, :width]
                    for c in range(3):
                        nc.tensor.matmul(ps_fc, s_rep[:, c, :],
                                         fc_sb[nm][:, c, :],
                                         start=(c == 0), stop=(c == 2))
                    gb = one.tile([TP, width], f32, name=f"gb_{nm}", tag=nm)
                    nc.scalar.activation(gb[:], ps_fc,
                                         mybir.ActivationFunctionType.Tanh)
                    gb_sb[nm] = gb
                nc.vector.tensor_tensor(gb_sb["wb1"][:], gb_sb["wb1"][:],
                                        b1c_sb[:], mybir.AluOpType.add)
                nc.vector.tensor_tensor(gb_sb["wb2"][:], gb_sb["wb2"][:],
                                        b2c_sb[:], mybir.AluOpType.add)

            with nc.named_scope("enc"):
                mv_all = persist.tile([TP, NT, 2], f32)
                std_all = persist.tile([TP, NT, 1], f32)
                rstd_all = persist.tile([TP, NT, 1], f32)
                nmr_all = persist.tile([TP, NT, 1], f32)
                for t in range(NT):
                    h_f = enc_p.tile([TP, HID], f32, tag="hf", name=f"h_{t}")
                    nc.vector.tensor_tensor(h_f[:], c1agg_sb[:, t, :],
                                            gb_sb["wg1"][:],
                                            mybir.AluOpType.mult)
                    nc.vector.tensor_tensor(h_f[:], h_f[:], gb_sb["wb1"][:],
                                            mybir.AluOpType.add)
                    nc.scalar.activation(c1agg_sb[:, t, :], h_f[:],
                                         mybir.ActivationFunctionType.Relu)
                for t in range(NT):
                    st6 = enc_p.tile([TP, 6], f32, tag="st6", name=f"st6_{t}")
                    nc.vector.bn_stats(st6[:], c1agg_sb[:, t, :])
                    nc.vector.bn_aggr(mv_all[:, t, :], st6[:])
                for t in range(NT):
                    nc.scalar.activation(std_all[:, t, :], mv_all[:, t, 1:2],
                                         mybir.ActivationFunctionType.Sqrt,
                                         bias=eps_sb[:, 0:1])
                for t in range(NT):
                    nc.vector.reciprocal(rstd_all[:, t, :], std_all[:, t, :])
                    nc.vector.scalar_tensor_tensor(
                        nmr_all[:, t, :], mv_all[:, t, 0:1], -1.0,
                        rstd_all[:, t, :],
                        mybir.AluOpType.mult, mybir.AluOpType.mult)
                for t in range(NT):
                    h1 = enc_p.tile([TP, HID], bf16, tag="h1", name=f"h1_{t}")
                    nc.scalar.activation(
                        h1[:], c1agg_sb[:, t, :],
                        mybir.ActivationFunctionType.Identity,
                        bias=nmr_all[:, t, 0:1], scale=rstd_all[:, t, 0:1])
                    h1T = enc_p.tile([TP, 2, TP], bf16, tag="h1T",
                                     name=f"h1T_{t}")
                    ps2w = ps_big.tile([TP, FUSED], f32, tag="big",
                                       name=f"w2_{t}")
                    ps2 = ps2w[:, :OUT]
                    ps_t = ps_sm.tile([TP, 2, TP], bf16, tag="smt",
                                      name=f"htr_{t}")
                    for k in range(2):
                        nc.tensor.transpose(ps_t[:, k, :],
                                            h1[:, k * TP:(k + 1) * TP],
                                            ident_sb[:])
                    nc.scalar.copy(h1T[:], ps_t[:])
                    for k in range(2):
                        nc.tensor.matmul(ps2, h1T[:, k, :], w2_sb[:, k, :],
                                         start=(k == 0), stop=(k == 1))
                    nc.vector.scalar_tensor_tensor(
                        tp_sb[:, t, :], ps2, dinv_sb[:, t:t + 1],
                        ones_sb[:, 0:1].to_broadcast((TP, OUT)),
                        mybir.AluOpType.mult, mybir.AluOpType.mult)
                    nc.sync.dma_start(
                        out=tsh_d.ap()[t * TP:(t + 1) * TP, :],
                        in_=tp_sb[:, t, :])
                with nc.named_scope("ag2"):
                    nc.gpsimd.collective_compute(
                        "AllGather", mybir.AluOpType.bypass, replica_groups=rg,
                        ins=[tsh_d.ap().opt()], outs=[tfull_d.ap().opt()])

            with nc.named_scope("pass2"):
                for t in range(NT):
                    ps_w = ps_agg.tile([TP, IN], f32, tag="agg",
                                       name=f"p2_{t}")
                    ps = ps_w[:, :OUT]
                    nc.tensor.matmul(ps, ident_sb[:], tp_sb[:, t, :],
                                     start=True, stop=False)
                    slot0 = 0
                    c0 = 0
                    for ci, n_i in enumerate(cfg.P2_CALLS):
                        base = cfg.P2_BASES[ci]
                        table = tfull_d.ap()[base:base + 32768, :]
                        g = gather(gat2, table,
                                   (t * cfg.SLOTS2 + slot0) // 16, n_i, OUT,
                                   bf16, f"g2_{t}_{ci}")
                        S = build_S(sbuild2, seg2_sb, bf16,
                                    t * cfg.CHUNKS2 + c0,
                                    n_i // TP, f"S2_{t}_{ci}")
                        for k in range(n_i // TP):
                            nc.tensor.matmul(
                                ps, S[:, k, :], g[:, k, :],
                                start=False,
                                stop=(ci == len(cfg.P2_CALLS) - 1
                                      and k == n_i // TP - 1))
                        slot0 += n_i
                        c0 += n_i // TP
                    dv = dinv_sb[:, t:t + 1]
                    o_f = epi.tile([TP, OUT], f32, tag="of", name=f"o_{t}")
                    nc.vector.scalar_tensor_tensor(
                        o_f[:], ps, dv, gb_sb["wg2"][:],
                        mybir.AluOpType.mult, mybir.AluOpType.mult)
                    nc.vector.tensor_tensor(o_f[:], o_f[:], gb_sb["wb2"][:],
                                            mybir.AluOpType.add)
                    st6 = small.tile([TP, 6], f32, tag="st6", name="st6")
                    mv = small.tile([TP, 2], f32, tag="mv", name="mv")
                    nc.vector.bn_stats(st6[:], o_f[:])
                    nc.vector.bn_aggr(mv[:], st6[:])
                    std = small.tile([TP, 1], f32, tag="std", name="std")
                    nc.scalar.activation(std[:], mv[:, 1:2],
                                         mybir.ActivationFunctionType.Sqrt,
                                         bias=eps_sb[:, 0:1])
                    rstd = small.tile([TP, 1], f32, tag="rstd", name="rstd")
                    nc.vector.reciprocal(rstd[:], std[:])
                    nmr = small.tile([TP, 1], f32, tag="nmr", name="nmr")
                    nc.vector.scalar_tensor_tensor(
                        nmr[:], mv[:, 0:1], -1.0, rstd[:],
                        mybir.AluOpType.mult, mybir.AluOpType.mult)
                    o_ln = epi.tile([TP, OUT], f32, tag="oln", name=f"ol_{t}")
                    nc.scalar.activation(
                        o_ln[:], o_f[:],
                        mybir.ActivationFunctionType.Identity,
                        bias=nmr[:, 0:1], scale=rstd[:, 0:1])
                    nc.sync.dma_start(out=out_d.ap()[t * TP:(t + 1) * TP, :],
                                      in_=o_ln[:])

    nc.compile()
    return nc



_CACHE = {}


def _get_program(cfg):
    key = (cfg.NT,)
    if key not in _CACHE:
        _CACHE[key] = build_program(cfg)
    return _CACHE[key]


def run(inputs, cfg=FULL, trace=False, **kw):
    nc = _get_program(cfg)
    in_maps = make_in_maps(inputs, cfg)
    res = bass_utils.run_bass_kernel_spmd(
        nc, in_maps, core_ids=list(range(cfg.NC)), trace=trace, **kw)
    out = np.concatenate([res.results[c]["out"] for c in range(cfg.NC)],
                         axis=0)[: cfg.n_real]
    return out.astype(np.float32), res


def kernel(**inputs):
    out, _ = run(inputs, FULL)
    return out


# revision 40
# speedup vs baseline: 1.0208x; 1.0208x over previous
import sys
import numpy as np
import ml_dtypes

sys.path.insert(0, "/opt/trn_rl_repo")

from concourse import bass, bacc, tile, mybir
from concourse import bass_utils

BF16 = ml_dtypes.bfloat16
F8 = ml_dtypes.float8_e4m3
dt = mybir.dt



class Cfg:
    def __init__(self, NT=49, n_real=50000):
        self.NC = 8
        self.TP = 128
        self.NT = NT
        self.IN = 256
        self.HID = 256
        self.OUT = 128
        self.FUSED = 512
        self.SHARD = NT * self.TP
        self.NPAD = self.NC * self.SHARD
        self.n_real = n_real
        self.LN_EPS = 1e-5
        self.P1_CALLS = [384, 1024, 1024]
        self.P1_BASES = [None, 0, 17408]
        self.P2_CALLS = [1024, 640, 512]
        self.P2_BASES = [0, 8704, 17408]
        self.SLOTS = 2432
        self.CHUNKS = self.SLOTS // self.TP
        self.SLOTS2 = sum(self.P2_CALLS)
        self.CHUNKS2 = self.SLOTS2 // self.TP


FULL = Cfg()



def _wrap16(vals, nrows=128):
    n = vals.shape[0]
    assert n % 16 == 0
    w = vals.reshape(n // 16, 16).T
    return np.tile(w, (nrows // 16, 1))


def _pmaj(vals, TP=128):
    return np.ascontiguousarray(vals.reshape(-1, TP).T)


def _pack_calls(s_sorted, d_sorted, calls, bases):
    total = sum(calls)
    idx = np.zeros(total, np.int64)
    seg = -np.ones(total, np.float32)
    pos = 0
    off = 0
    n = s_sorted.shape[0]
    for cap, base in zip(calls, bases):
        take = 0
        limit = base + 32768
        while take < cap and pos + take < n and s_sorted[pos + take] < limit:
            take += 1
        s_blk = s_sorted[pos:pos + take]
        if take and s_blk[0] < base:
            raise OverflowError(f"src {s_blk[0]} below window base {base}")
        idx[off:off + take] = s_blk - base
        seg[off:off + take] = d_sorted[pos:pos + take]
        pos += take
        off += cap
    if pos != n:
        raise OverflowError(f"failed to pack: {n - pos} messages left over")
    return idx, seg


def preprocess(edge_index, cfg):
    src = np.asarray(edge_index[0], dtype=np.int64)
    dst = np.asarray(edge_index[1], dtype=np.int64)
    deg = np.bincount(src, minlength=cfg.NPAD).astype(np.float32) + 1.0

    per_core = []
    shard_of = dst // cfg.SHARD
    for c in range(cfg.NC):
        m = shard_of == c
        s_c = src[m]
        d_c = dst[m] - c * cfg.SHARD
        tile_of = d_c // cfg.TP
        order = np.argsort(tile_of, kind="stable")
        s_c, d_c, tile_of = s_c[order], d_c[order], tile_of[order]
        bounds = np.searchsorted(tile_of, np.arange(cfg.NT + 1))

        lo, hi = c * cfg.SHARD, (c + 1) * cfg.SHARD
        cap0 = cfg.P1_CALLS[0]
        idx1 = np.zeros((cfg.NT, cfg.SLOTS), np.int64)
        seg1 = -np.ones((cfg.NT, cfg.SLOTS), np.float32)
        idx2 = np.zeros((cfg.NT, cfg.SLOTS2), np.int64)
        seg2 = -np.ones((cfg.NT, cfg.SLOTS2), np.float32)
        for t in range(cfg.NT):
            sl = slice(bounds[t], bounds[t + 1])
            s_t = s_c[sl]
            d_t = d_c[sl] - t * cfg.TP
            o = np.argsort(s_t, kind="stable")
            s_t, d_t = s_t[o], d_t[o]

            own = (s_t >= lo) & (s_t < hi)
            s_o, d_o = s_t[own], d_t[own]
            n_o = s_o.shape[0]
            if n_o > cap0:
                raise OverflowError(f"own overflow {n_o}")
            idx1[t, :n_o] = s_o - lo
            seg1[t, :n_o] = d_o
            s_x, d_x = s_t[~own], d_t[~own]
            ix, sg = _pack_calls(s_x, d_x, cfg.P1_CALLS[1:], cfg.P1_BASES[1:])
            idx1[t, cap0:] = ix
            seg1[t, cap0:] = sg

            ix, sg = _pack_calls(s_t, d_t, cfg.P2_CALLS, cfg.P2_BASES)
            idx2[t] = ix
            seg2[t] = sg

        def segmaj(a):
            return np.ascontiguousarray(a.reshape(-1, cfg.TP).T)

        idx1r = np.concatenate([idx1[:, :cap0].reshape(-1),
                                idx1[:, cap0:].reshape(-1)])
        seg1r = np.concatenate([segmaj(seg1[:, :cap0]),
                                segmaj(seg1[:, cap0:])], axis=1)
        per_core.append({
            "idx1": _wrap16(idx1r).astype(np.int16),
            "idx2": _wrap16(idx2.reshape(-1)).astype(np.int16),
            "seg1": np.ascontiguousarray(seg1r).astype(BF16),
            "seg2": segmaj(seg2).astype(BF16),
        })
    return deg, per_core


def make_in_maps(inputs, cfg):
    x = np.asarray(inputs["x"], np.float32)
    deg, per_core = preprocess(np.asarray(inputs["edge_index"]), cfg)

    xp = np.zeros((cfg.NPAD, cfg.IN), np.float32)
    xp[: x.shape[0]] = x
    dinv = deg ** -0.5
    xf8 = (dinv[:, None] * xp).astype(F8)

    def chunks(a, k):
        return np.ascontiguousarray(a.reshape(k, 128, a.shape[1]))

    wf = np.concatenate([np.asarray(inputs["sig_conv_w"], np.float32),
                         np.asarray(inputs["conv1_w"], np.float32)], axis=1)

    def aug(w, b):
        wt = np.asarray(w, np.float32).T
        a = np.zeros((3 * 128, wt.shape[1]), np.float32)
        a[: wt.shape[0]] = wt
        a[wt.shape[0]] = np.asarray(b, np.float32)
        return chunks(a, 3)

    shared = {
        "wf": chunks(wf, 2).astype(BF16),
        "w2": chunks(np.asarray(inputs["conv2_w"], np.float32), 2).astype(BF16),
        "wg1": aug(inputs["fc1_w"], inputs["fc1_b"]),
        "wb1": aug(inputs["fc2_w"], inputs["fc2_b"]),
        "wg2": aug(inputs["fc3_w"], inputs["fc3_b"]),
        "wb2": aug(inputs["fc4_w"], inputs["fc4_b"]),
        "bsig": np.broadcast_to(np.asarray(inputs["sig_conv_b"], np.float32),
                                (128, cfg.HID)).copy(),
        "b1c": np.broadcast_to(np.asarray(inputs["conv1_b"], np.float32),
                               (128, cfg.HID)).copy(),
        "b2c": np.broadcast_to(np.asarray(inputs["conv2_b"], np.float32),
                               (128, cfg.OUT)).copy(),
        "iota": np.broadcast_to(np.arange(128, dtype=np.float32),
                                (128, 128)).astype(BF16).copy(),
        "ident": np.eye(128, dtype=np.float32).astype(BF16),
        "identf8": np.eye(128, dtype=np.float32).astype(F8),
    }

    in_maps = []
    node_ids = np.arange(cfg.SHARD)
    for c in range(cfg.NC):
        sl = slice(c * cfg.SHARD, (c + 1) * cfg.SHARD)
        gids = node_ids + c * cfg.SHARD
        m = dict(shared)
        m["xsh"] = np.ascontiguousarray(xf8[sl])
        m["xnp"] = np.ascontiguousarray(
            xf8[sl].reshape(cfg.NT, cfg.TP, cfg.IN).transpose(1, 0, 2))
        m["deg"] = _pmaj(deg[sl]).copy()
        m["sigmask"] = _pmaj((gids < cfg.n_real).astype(np.float32)).astype(BF16)
        m.update(per_core[c])
        in_maps.append(m)
    return in_maps



def build_program(cfg):
    nc = bacc.Bacc("TRN2", target_bir_lowering=False, debug=False,
                   num_devices=cfg.NC, num_swdge_queues=4)
    f32, bf16, i16, f8 = dt.float32, dt.bfloat16, dt.int16, dt.float8e4
    TP, NT = cfg.TP, cfg.NT
    IN, HID, OUT, FUSED = cfg.IN, cfg.HID, cfg.OUT, cfg.FUSED
    CH = cfg.CHUNKS
    ICOLS = cfg.SLOTS * NT // 16
    SCOLS = CH * NT

    def inp(name, shape, dtype):
        return nc.dram_tensor(name, shape, dtype, kind="ExternalInput")

    xsh_d = inp("xsh", [cfg.SHARD, IN], f8)
    xnp_d = inp("xnp", [TP, NT, IN], f8)
    wf_d = inp("wf", [2, TP, FUSED], bf16)
    w2_d = inp("w2", [2, TP, OUT], bf16)
    wg1_d = inp("wg1", [3, TP, HID], f32)
    wb1_d = inp("wb1", [3, TP, HID], f32)
    wg2_d = inp("wg2", [3, TP, OUT], f32)
    wb2_d = inp("wb2", [3, TP, OUT], f32)
    bsig_d = inp("bsig", [TP, HID], f32)
    b1c_d = inp("b1c", [TP, HID], f32)
    b2c_d = inp("b2c", [TP, OUT], f32)
    iota_d = inp("iota", [TP, TP], bf16)
    ident_d = inp("ident", [TP, TP], bf16)
    identf8_d = inp("identf8", [TP, TP], f8)
    deg_d = inp("deg", [TP, NT], f32)
    mask_d = inp("sigmask", [TP, NT], bf16)
    idx1_d = inp("idx1", [TP, ICOLS], i16)
    idx2_d = inp("idx2", [TP, cfg.SLOTS2 * NT // 16], i16)
    seg1_d = inp("seg1", [TP, SCOLS], bf16)
    seg2_d = inp("seg2", [TP, cfg.CHUNKS2 * NT], bf16)

    out_d = nc.dram_tensor("out", [cfg.SHARD, OUT], f32, kind="ExternalOutput")

    xin_d = nc.dram_tensor("xin", [cfg.SHARD, IN], f8)
    xfull_d = nc.dram_tensor("xfull", [cfg.NPAD, IN], f8,
                             addr_space="Shared")
    tsh_d = nc.dram_tensor("tsh", [cfg.SHARD, OUT], bf16)
    tfull_d = nc.dram_tensor("tfull", [cfg.NPAD, OUT], bf16,
                             addr_space="Shared")
    sin_d = nc.dram_tensor("sin", [1, HID], f32)
    sout_d = nc.dram_tensor("sout", [1, HID], f32, addr_space="Shared")

    rg = [list(range(cfg.NC))]
    qctr = [0]

    with tile.TileContext(nc) as tc:
        with (
            tc.tile_pool(name="const", bufs=1) as const,
            tc.tile_pool(name="persist", bufs=1) as persist,
            tc.tile_pool(name="gat", bufs=6) as gat,
            tc.tile_pool(name="gat2", bufs=4) as gat2,
            tc.tile_pool(name="gato", bufs=6) as gato,
            tc.tile_pool(name="sbuild", bufs=6) as sbuild,
            tc.tile_pool(name="sbuild2", bufs=4) as sbuild2,
            tc.tile_pool(name="epi", bufs=3) as epi,
            tc.tile_pool(name="small", bufs=4) as small,
            tc.tile_pool(name="enc_p", bufs=8) as enc_p,
            tc.tile_pool(name="one", bufs=1) as one,
            tc.tile_pool(name="ps_agg", bufs=3, space="PSUM") as ps_agg,
            tc.tile_pool(name="ps_big", bufs=2, space="PSUM") as ps_big,
            tc.tile_pool(name="ps_sig", bufs=1, space="PSUM") as ps_sig,
            tc.tile_pool(name="ps_sm", bufs=2, space="PSUM") as ps_sm,
        ):
            idxA_sb = const.tile([TP, ICOLS], i16)
            seg1_sb = const.tile([TP, SCOLS], bf16)
            seg2_sb = const.tile([TP, cfg.CHUNKS2 * NT], bf16)
            iota_sb = const.tile([TP, TP], bf16)
            ident_sb = const.tile([TP, TP], bf16)
            identf8_sb = const.tile([TP, TP], f8)
            deg_sb = const.tile([TP, NT], f32)
            mask_sb = const.tile([TP, NT], bf16)
            OCOLS = NT * cfg.P1_CALLS[0] // 16
            nc.sync.dma_start(out=xin_d.ap(), in_=xsh_d.ap())
            nc.sync.dma_start(out=idxA_sb[:, :OCOLS],
                              in_=idx1_d.ap()[:, :OCOLS])
            for t_, d in ((seg1_sb, seg1_d), (iota_sb, iota_d),
                          (identf8_sb, identf8_d), (deg_sb, deg_d)):
                nc.sync.dma_start(out=t_[:], in_=d.ap())
            xn_sb = persist.tile([TP, NT, IN], f8)
            ownp_sb = persist.tile([TP, NT, IN], bf16)
            c1agg_sb = persist.tile([TP, NT, HID], bf16)
            tp_sb = persist.tile([TP, NT, OUT], bf16)
            nc.sync.dma_start(out=xn_sb[:], in_=xnp_d.ap())
            nc.sync.dma_start(out=idxA_sb[:, OCOLS:],
                              in_=idx1_d.ap()[:, OCOLS:])

            wf_sb = const.tile([TP, 2, FUSED], bf16)
            w2_sb = const.tile([TP, 2, OUT], bf16)
            nc.sync.dma_start(out=wf_sb[:], in_=wf_d.ap().transpose([1, 0, 2]))
            nc.sync.dma_start(out=w2_sb[:], in_=w2_d.ap().transpose([1, 0, 2]))
            fc_sb = {}
            for nm, d, width in (("wg1", wg1_d, HID), ("wb1", wb1_d, HID),
                                 ("wg2", wg2_d, OUT), ("wb2", wb2_d, OUT)):
                t_ = const.tile([TP, 3, width], f32, name=nm)
                nc.sync.dma_start(out=t_[:], in_=d.ap().transpose([1, 0, 2]))
                fc_sb[nm] = t_
            bsig_sb = const.tile([TP, HID], f32)
            b1c_sb = const.tile([TP, HID], f32)
            b2c_sb = const.tile([TP, OUT], f32)
            for t_, d in ((bsig_sb, bsig_d), (b1c_sb, b1c_d),
                          (b2c_sb, b2c_d), (seg2_sb, seg2_d),
                          (mask_sb, mask_d), (ident_sb, ident_d)):
                nc.sync.dma_start(out=t_[:], in_=d.ap())

            eps_sb = const.tile([TP, 1], f32)
            nc.vector.memset(eps_sb[:], cfg.LN_EPS)
            ones_sb = const.tile([TP, 1], f32)
            nc.vector.memset(ones_sb[:], 1.0)
            dinv_sb = const.tile([TP, NT], f32)
            nc.scalar.sqrt(dinv_sb[:], deg_sb[:])
            nc.vector.reciprocal(dinv_sb[:], dinv_sb[:])

            def build_S(pool, seg_src, sdt, scol, nch, name):
                S = pool.tile([TP, nch, TP], sdt,
                              tag=f"S{nch}_{sdt}", name=name)
                cols = slice(scol, scol + nch)
                nc.vector.tensor_tensor(
                    S[:],
                    seg_src[:, cols].unsqueeze(2).to_broadcast((TP, nch, TP)),
                    iota_sb[:].unsqueeze(1).to_broadcast((TP, nch, TP)),
                    mybir.AluOpType.is_equal)
                return S

            def gather(pool, table_ap, icol0, n_idx, width, gdt, name):
                g = pool.tile([TP, n_idx // TP, width], gdt,
                              tag=f"g{n_idx}_{width}_{gdt}", name=name)
                nc.gpsimd.dma_gather(
                    out_ap=g[:],
                    in_ap=table_ap,
                    idxs_ap=idxA_sb[:, icol0:icol0 + n_idx // 16],
                    num_idxs=n_idx,
                    num_idxs_reg=n_idx,
                    elem_size=width,
                    queue_num=qctr[0] % 4,
                )
                qctr[0] += 1
                return g

            with nc.named_scope("own1"):
                for t in range(NT):
                    n_o = cfg.P1_CALLS[0]
                    g = gather(gato, xsh_d.ap(), t * (n_o // 16), n_o, IN,
                               f8, f"go_{t}")
                    S = build_S(sbuild, seg1_sb, f8, t * (n_o // TP),
                                n_o // TP, f"So_{t}")
                    ps = ps_agg.tile([TP, IN], f32, tag="agg", name=f"po_{t}")
                    nc.tensor.matmul(ps[:], identf8_sb[:], xn_sb[:, t, :],
                                     start=True, stop=False)
                    for k in range(n_o // TP):
                        nc.tensor.matmul(ps[:], S[:, k, :], g[:, k, :],
                                         start=False,
                                         stop=(k == n_o // TP - 1))
                    nc.scalar.copy(ownp_sb[:, t, :], ps[:])

            with nc.named_scope("ag1"):
                nc.gpsimd.collective_compute(
                    "AllGather", mybir.AluOpType.bypass, replica_groups=rg,
                    ins=[xin_d.ap().opt()], outs=[xfull_d.ap().opt()])

            s_ps = ps_sig.tile([1, HID], f32)
            with nc.named_scope("cross1"):
                for t in range(NT):
                    ps = ps_agg.tile([TP, IN], f32, tag="agg", name=f"pa_{t}")
                    n_own = cfg.P1_CALLS[0]
                    nco = sum(cfg.P1_CALLS[1:])
                    ib = NT * n_own // 16 + t * nco // 16
                    sb_ = NT * (n_own // TP) + t * (nco // TP)
                    slot0 = 0
                    c0 = 0
                    first = True
                    for ci, n_i in enumerate(cfg.P1_CALLS[1:]):
                        base = cfg.P1_BASES[1 + ci]
                        table = xfull_d.ap()[base:base + 32768, :]
                        g = gather(gat, table, ib + slot0 // 16, n_i, IN, f8,
                                   f"gx_{t}_{ci}")
                        S = build_S(sbuild, seg1_sb, f8, sb_ + c0,
                                    n_i // TP, f"Sx_{t}_{ci}")
                        for k in range(n_i // TP):
                            nc.tensor.matmul(
                                ps[:], S[:, k, :], g[:, k, :],
                                start=first,
                                stop=(ci == len(cfg.P1_CALLS) - 2
                                      and k == n_i // TP - 1))
                            first = False
                        slot0 += n_i
                        c0 += n_i // TP
                    agg_b = epi.tile([TP, IN], bf16, tag="aggb",
                                     name=f"ab_{t}")
                    nc.vector.tensor_tensor(agg_b[:], ps[:],
                                            ownp_sb[:, t, :],
                                            mybir.AluOpType.add)
                    aggT = epi.tile([TP, 2, TP], bf16, tag="aggT",
                                    name=f"at_{t}")
                    ps_t = ps_sm.tile([TP, 2, TP], bf16, tag="smt",
                                      name=f"tr_{t}")
                    for k in range(2):
                        nc.tensor.transpose(ps_t[:, k, :],
                                            agg_b[:, k * TP:(k + 1) * TP],
                                            ident_sb[:])
                    nc.scalar.copy(aggT[:], ps_t[:])
                    po = ps_big.tile([TP, FUSED], f32, tag="big",
                                     name=f"pw_{t}")
                    for k in range(2):
                        nc.tensor.matmul(po[:], aggT[:, k, :], wf_sb[:, k, :],
                                         start=(k == 0), stop=(k == 1))
                    dv = dinv_sb[:, t:t + 1]
                    sig_f = epi.tile([TP, HID], f32, tag="sigf",
                                     name=f"sf_{t}")
                    nc.vector.scalar_tensor_tensor(
                        sig_f[:], po[:, :HID], dv, bsig_sb[:],
                        mybir.AluOpType.mult, mybir.AluOpType.add)
                    sig_b = epi.tile([TP, HID], bf16, tag="sigb",
                                     name=f"sb_{t}")
                    nc.scalar.activation(sig_b[:], sig_f[:],
                                         mybir.ActivationFunctionType.Relu)
                    nc.tensor.matmul(s_ps[:], mask_sb[:, t:t + 1], sig_b[:],
                                     start=(t == 0), stop=(t == NT - 1))
                    nc.scalar.activation(c1agg_sb[:, t, :], po[:, HID:],
                                         mybir.ActivationFunctionType.Copy,
                                         scale=dv)

            with nc.named_scope("sig"):
                s_sb = one.tile([1, HID], f32)
                nc.scalar.copy(s_sb[:], s_ps[:])
                nc.sync.dma_start(out=sin_d.ap(), in_=s_sb[:])
                nc.gpsimd.collective_compute(
                    "AllReduce", mybir.AluOpType.add, replica_groups=rg,
                    ins=[sin_d.ap().opt()], outs=[sout_d.ap().opt()])
                nc.sync.dma_start(
                    out=idxA_sb[:, :cfg.SLOTS2 * NT // 16],
                    in_=idx2_d.ap())

                s_col = one.tile([TP, 3], f32)
                nc.vector.memset(s_col[:], 0.0)
                nc.vector.memset(s_col[0:1, 2:3], 1.0)
                nc.sync.dma_start(
                    out=s_col[:, 0:2],
                    in_=sout_d.ap().rearrange("o (c p) -> (o c) p", p=TP)
                        .transpose([1, 0]))
                s_rep = one.tile([TP, 3, TP], f32)
                for c in range(3):
                    nc.vector.tensor_copy(
                        s_rep[:, c, :],
                        s_col[:, c:c + 1].to_broadcast((TP, TP)))
                gb_sb = {}
                for nm, width in (("wg1", HID), ("wb1", HID),
                                  ("wg2", OUT), ("wb2", OUT)):
                    ps_fcw = ps_big.tile([TP, FUSED], f32, tag="big",
                                         name=nm)
                    ps_fc = ps_fcw[:, :width]
                    for c in range(3):
                        nc.tensor.matmul(ps_fc, s_rep[:, c, :],
                                         fc_sb[nm][:, c, :],
                                         start=(c == 0), stop=(c == 2))
                    gb = one.tile([TP, width], f32, name=f"gb_{nm}", tag=nm)
                    nc.scalar.activation(gb[:], ps_fc,
                                         mybir.ActivationFunctionType.Tanh)
                    gb_sb[nm] = gb
                nc.vector.tensor_tensor(gb_sb["wb1"][:], gb_sb["wb1"][:],
                                        b1c_sb[:], mybir.AluOpType.add)
                nc.vector.tensor_tensor(gb_sb["wb2"][:], gb_sb["wb2"][:],
                                        b2c_sb[:], mybir.AluOpType.add)

            with nc.named_scope("enc"):
                mv_all = persist.tile([TP, NT, 2], f32)
                std_all = persist.tile([TP, NT, 1], f32)
                rstd_all = persist.tile([TP, NT, 1], f32)
                nmr_all = persist.tile([TP, NT, 1], f32)
                for t in range(NT):
                    h_f = enc_p.tile([TP, HID], f32, tag="hf", name=f"h_{t}")
                    nc.vector.tensor_tensor(h_f[:], c1agg_sb[:, t, :],
                                            gb_sb["wg1"][:],
                                            mybir.AluOpType.mult)
                    nc.vector.tensor_tensor(h_f[:], h_f[:], gb_sb["wb1"][:],
                                            mybir.AluOpType.add)
                    nc.scalar.activation(c1agg_sb[:, t, :], h_f[:],
                                         mybir.ActivationFunctionType.Relu)
                for t in range(NT):
                    st6 = enc_p.tile([TP, 6], f32, tag="st6", name=f"st6_{t}")
                    nc.vector.bn_stats(st6[:], c1agg_sb[:, t, :])
                    nc.vector.bn_aggr(mv_all[:, t, :], st6[:])
                for t in range(NT):
                    nc.scalar.activation(std_all[:, t, :], mv_all[:, t, 1:2],
                                         mybir.ActivationFunctionType.Sqrt,
                                         bias=eps_sb[:, 0:1])
                for t in range(NT):
                    nc.vector.reciprocal(rstd_all[:, t, :], std_all[:, t, :])
                    nc.vector.scalar_tensor_tensor(
                        nmr_all[:, t, :], mv_all[:, t, 0:1], -1.0,
                        rstd_all[:, t, :],
                        mybir.AluOpType.mult, mybir.AluOpType.mult)
                for t in range(NT):
                    h1 = enc_p.tile([TP, HID], bf16, tag="h1", name=f"h1_{t}")
                    nc.scalar.activation(
                        h1[:], c1agg_sb[:, t, :],
                        mybir.ActivationFunctionType.Identity,
                        bias=nmr_all[:, t, 0:1], scale=rstd_all[:, t, 0:1])
                    h1T = enc_p.tile([TP, 2, TP], bf16, tag="h1T",
                                     name=f"h1T_{t}")
                    ps2w = ps_big.tile([TP, FUSED], f32, tag="big",
                                       name=f"w2_{t}")
                    ps2 = ps2w[:, :OUT]
                    ps_t = ps_sm.tile([TP, 2, TP], bf16, tag="smt",
                                      name=f"htr_{t}")
                    for k in range(2):
                        nc.tensor.transpose(ps_t[:, k, :],
                                            h1[:, k * TP:(k + 1) * TP],
                                            ident_sb[:])
                    nc.scalar.copy(h1T[:], ps_t[:])
                    for k in range(2):
                        nc.tensor.matmul(ps2, h1T[:, k, :], w2_sb[:, k, :],
                                         start=(k == 0), stop=(k == 1))
                    nc.vector.scalar_tensor_tensor(
                        tp_sb[:, t, :], ps2, dinv_sb[:, t:t + 1],
                        ones_sb[:, 0:1].to_broadcast((TP, OUT)),
                        mybir.AluOpType.mult, mybir.AluOpType.mult)
                    nc.sync.dma_start(
                        out=tsh_d.ap()[t * TP:(t + 1) * TP, :],
                        in_=tp_sb[:, t, :])
                with nc.named_scope("ag2"):
                    nc.gpsimd.collective_compute(
                        "AllGather", mybir.AluOpType.bypass, replica_groups=rg,
                        ins=[tsh_d.ap().opt()], outs=[tfull_d.ap().opt()])

            with nc.named_scope("pass2"):
                for t in range(NT):
                    ps_w = ps_agg.tile([TP, IN], f32, tag="agg",
                                       name=f"p2_{t}")
                    ps = ps_w[:, :OUT]
                    nc.tensor.matmul(ps, ident_sb[:], tp_sb[:, t, :],
                                     start=True, stop=False)
                    slot0 = 0
                    c0 = 0
                    for ci, n_i in enumerate(cfg.P2_CALLS):
                        base = cfg.P2_BASES[ci]
                        table = tfull_d.ap()[base:base + 32768, :]
                        g = gather(gat2, table,
                                   (t * cfg.SLOTS2 + slot0) // 16, n_i, OUT,
                                   bf16, f"g2_{t}_{ci}")
                        S = build_S(sbuild2, seg2_sb, bf16,
                                    t * cfg.CHUNKS2 + c0,
                                    n_i // TP, f"S2_{t}_{ci}")
                        for k in range(n_i // TP):
                            nc.tensor.matmul(
                                ps, S[:, k, :], g[:, k, :],
                                start=False,
                                stop=(ci == len(cfg.P2_CALLS) - 1
                                      and k == n_i // TP - 1))
                        slot0 += n_i
                        c0 += n_i // TP
                    dv = dinv_sb[:, t:t + 1]
                    o_f = epi.tile([TP, OUT], f32, tag="of", name=f"o_{t}")
                    nc.vector.scalar_tensor_tensor(
                        o_f[:], ps, dv, gb_sb["wg2"][:],
                        mybir.AluOpType.mult, mybir.AluOpType.mult)
                    nc.vector.tensor_tensor(o_f[:], o_f[:], gb_sb["wb2"][:],
                                            mybir.AluOpType.add)
                    st6 = small.tile([TP, 6], f32, tag="st6", name="st6")
                    mv = small.tile([TP, 2], f32, tag="mv", name="mv")
                    nc.vector.bn_stats(st6[:], o_f[:])
                    nc.vector.bn_aggr(mv[:], st6[:])
                    std = small.tile([TP, 1], f32, tag="std", name="std")
                    nc.scalar.activation(std[:], mv[:, 1:2],
                                         mybir.ActivationFunctionType.Sqrt,
                                         bias=eps_sb[:, 0:1])
                    rstd = small.tile([TP, 1], f32, tag="rstd", name="rstd")
                    nc.vector.reciprocal(rstd[:], std[:])
                    nmr = small.tile([TP, 1], f32, tag="nmr", name="nmr")
                    nc.vector.scalar_tensor_tensor(
                        nmr[:], mv[:, 0:1], -1.0, rstd[:],
                        mybir.AluOpType.mult, mybir.AluOpType.mult)
                    o_ln = epi.tile([TP, OUT], f32, tag="oln", name=f"ol_{t}")
                    nc.scalar.activation(
                        o_ln[:], o_f[:],
                        mybir.ActivationFunctionType.Identity,
                        bias=nmr[:, 0:1], scale=rstd[:, 0:1])
                    nc.sync.dma_start(out=out_d.ap()[t * TP:(t + 1) * TP, :],
                                      in_=o_ln[:])

    nc.compile()
    return nc



_CACHE = {}


def _get_program(cfg):
    key = (cfg.NT,)
    if key not in _CACHE:
        _CACHE[key] = build_program(cfg)
    return _CACHE[key]


def run(inputs, cfg=FULL, trace=False, **kw):
    nc = _get_program(cfg)
    in_maps = make_in_maps(inputs, cfg)
    res = bass_utils.run_bass_kernel_spmd(
        nc, in_maps, core_ids=list(range(cfg.NC)), trace=trace, **kw)
    out = np.concatenate([res.results[c]["out"] for c in range(cfg.NC)],
                         axis=0)[: cfg.n_real]
    return out.astype(np.float32), res


def kernel(**inputs):
    out, _ = run(inputs, FULL)
    return out
